# revision 16
# baseline (speedup 1.0000x reference)
"""Causal attention (B=8, S=2048, D=1024, fp32) on 8 TRN2 NeuronCores.

Sharding: batch-parallel, one batch element per core (SPMD, no collectives).

Per-core algorithm (S^T layout):
  - Host casts Q/K/V to bf16 (identical rounding to an in-DMA cast), halving
    input HBM traffic; output is stored bf16 and upcast on host.
  - Q, K tiles are DMA'd to staging (Q on the sync queue, K/V on gpsimd, so
    issue costs overlap) and transposed on TensorE (128x128 tiles vs a bf16
    identity) into [d, s] layouts QT/KT.  From group 2 on, the next group's
    transposes are interleaved into the current group's pair loop so no
    transpose block (and its exposed LDWEIGHTS) sits between groups.
  - Scores are computed transposed: S^T[k, q] = sum_d KT[d,k] * QT[d,q],
    accumulated over 8 d-subtiles in PSUM, 2 k-tiles x 256 q per PSUM bank;
    k-tiles above the diagonal are skipped entirely.
  - Causal mask: multiplicative bf16 mask on the diagonal pair only, applied
    to P^T after exp; the fully-masked quarter is memset, never computed.
  - exp(dots/sqrt(D)) on ScalarE (no max subtraction: |dots| <= ~1.1e3 so
    logits <= ~35, exp fits fp32 comfortably), output cast to bf16 = P^T.
  - Row sums: P^T is accumulated across pairs on DVE into fp32, folded to
    bf16 at group end, and two tiny ones-matmuls give per-q sums.  (This
    replaces one N=1 ones-matmul per pair/kk/j whose successor LDWEIGHTS
    was ~127 exposed PE cycles each.)
  - PV: O[q, d] += P^T.T @ V with V in native [k, d] layout; normalization
    is a DVE/ACT multiply by the reciprocal row sum (numerator/denominator
    both derive from the same bf16 P^T, so rounding largely cancels).
  - Dummy matmuls at t~0 keep the PE busy during the DMA-bound startup so
    the HAM clock gate ramps to 8/8 before the real work lands; the causal
    mask is built only after the startup DMAs are issued.

Rejected alternatives (measured): XBAR dma_start_transpose runs at ~52 GB/s
aggregate (256B packets) -- starves the PE whether used for all or half the
tiles; fp8 (the only >1 MAC/cycle/PE path) fails the 2e-2 error budget;
QK with N=512 rhs needs 512-wide PV accumulators = all 8 PSUM banks.
"""

import numpy as np

import concourse.bass as bass
import concourse.mybir as mybir
import concourse.tile as tile
from concourse import bacc
from concourse.masks import make_identity

P = 128


def build_attention_nc(S=2048, D=1024):
    f32, bf16 = mybir.dt.float32, mybir.dt.bfloat16
    nc = bacc.Bacc(None, target_bir_lowering=False)

    q_d = nc.dram_tensor("query", [S, D], bf16, kind="ExternalInput")
    k_d = nc.dram_tensor("key", [S, D], bf16, kind="ExternalInput")
    v_d = nc.dram_tensor("value", [S, D], bf16, kind="ExternalInput")
    o_d = nc.dram_tensor("out", [S, D], bf16, kind="ExternalOutput")

    NT = S // P            # number of 128-row seq tiles
    ND = D // P            # number of 128-wide d subtiles
    QGT = 2                # q-tiles per group
    QG = QGT * P           # q-group width (256)
    NG = S // QG           # number of q groups
    DH = min(D, 512)       # PV free-dim chunk (one PSUM bank)
    NDH = D // DH
    TCH = ND if ND <= 8 else 4  # transpose chunk (one copy per tile)
    scale = 1.0 / float(np.sqrt(D))

    qv = q_d.rearrange("(n p) d -> p n d", p=P)
    kv = k_d.rearrange("(n p) d -> p n d", p=P)
    vv = v_d.rearrange("(n p) d -> p n d", p=P)
    ov = o_d.rearrange("(n p) d -> p n d", p=P)

    with tile.TileContext(nc) as tc:
        with (
            tc.tile_pool(name="const", bufs=1) as constp,
            tc.tile_pool(name="slab", bufs=1) as slab,
            tc.tile_pool(name="stage", bufs=8) as stagep,
            tc.tile_pool(name="pt", bufs=3) as ptp,
            tc.tile_pool(name="psum_sb", bufs=2) as psumsb,
            tc.tile_pool(name="small", bufs=2) as smallp,
            tc.tile_pool(name="ost", bufs=2) as ostp,
            tc.tile_pool(name="ps", bufs=1, space="PSUM") as psp,
        ):
            ident = constp.tile([P, P], bf16)
            make_identity(nc, ident[:])
            ones = constp.tile([P, 1], bf16)
            nc.vector.memset(ones[:], 1.0)
            warmslab = constp.tile([P, 512], bf16)
            nc.vector.memset(warmslab[:], 0.0)

            QT = slab.tile([P, ND, S], bf16)   # [d%128, d//128, q]
            KT = slab.tile([P, ND, S], bf16)   # [d%128, d//128, k]
            V = slab.tile([P, NT, D], bf16)    # [k%128, k//128, d]

            # Warm the PE clock gate (HAM) during the DMA-bound startup:
            # dep-free matmuls on a memset slab keep the PE streaming until
            # the first real transposes arrive.
            for _ in range(8):
                warm = psp.tile([P, 512], f32, tag="st", bufs=3)
                nc.tensor.matmul(
                    warm[:], lhsT=warmslab[:, :P], rhs=warmslab[:],
                    start=True, stop=True,
                )

            def emit_loads(g, fine=False):
                """Issue the DMAs for group g's new Q/K/V tiles.

                Q goes out on the sync queue, K/V on gpsimd, so the issue
                costs overlap.  fine=True splits tiles into half-D chunks so
                the first subtile transposes start as soon as half a tile
                has landed (startup only).
                """
                stages = {}
                for nm, srcv, eng in (("q", qv, nc.sync), ("k", kv, nc.gpsimd)):
                    for t in range(QGT * g, QGT * (g + 1)):
                        stg = stagep.tile([P, D], bf16, tag="stage", name=f"stg_{nm}{t}")
                        if fine:
                            hd = D // 2
                            eng.dma_start(stg[:, :hd], srcv[:, t, :hd])
                            eng.dma_start(stg[:, hd:], srcv[:, t, hd:])
                        else:
                            eng.dma_start(stg[:], srcv[:, t, :])
                        stages[(nm, t)] = stg
                for t in range(QGT * g, QGT * (g + 1)):
                    nc.gpsimd.dma_start(V[:, t, :], vv[:, t, :])
                return stages

            def transpose_unit(nm, dst, t, stages):
                stg = stages[(nm, t)]
                for c in range(ND // TCH):
                    pst = psp.tile([P, TCH, P], bf16, tag="st", bufs=3)
                    for j in range(TCH):
                        ds = c * TCH + j
                        nc.tensor.transpose(
                            pst[:, j, :],
                            stg[:, ds * P : (ds + 1) * P],
                            ident[:],
                        )
                    dslc = dst[:, c * TCH : (c + 1) * TCH, t * P : (t + 1) * P]
                    # Alternate copy engines by tile parity so the two tiles'
                    # copies overlap (DVE + ACT) instead of serializing.
                    if t % 2 == 0:
                        nc.vector.tensor_copy(dslc, pst[:])
                    else:
                        nc.scalar.copy(dslc, pst[:])

            def transpose_units(g):
                # Q tiles first: group g's QK matmuls need QT immediately,
                # but the new KT tiles only at the diagonal (last) pair.
                for nm, dst in (("q", QT), ("k", KT)):
                    for t in range(QGT * g, QGT * (g + 1)):
                        yield (nm, dst, t)

            def emit_transposes(g, stages):
                for nm, dst, t in transpose_units(g):
                    transpose_unit(nm, dst, t, stages)

            pending = emit_loads(0, fine=True)
            transposed_early = False

            # Multiplicative bf16 causal mask for the diagonal k-tile pair,
            # S^T layout (1=valid, 0=masked), applied to P^T after exp.
            # Built only now so the startup DMAs issue first.
            mask01 = constp.tile([P, 2, QG], bf16)
            for half in range(2):
                m = mask01[:, half, :]
                nc.gpsimd.memset(m, 1.0)
                nc.gpsimd.affine_select(
                    out=m,
                    in_=m,
                    compare_op=mybir.AluOpType.is_ge,
                    fill=0.0,
                    base=-(P * half),
                    pattern=[[1, QG]],
                    channel_multiplier=-1,
                )

            for g in range(NG):
                # Prefetch next group's DMA loads before anything else so
                # they land while this group's pair loop runs.
                nxt = emit_loads(g + 1, fine=(g == 0)) if g + 1 < NG else None
                if not transposed_early:
                    emit_transposes(g, pending)
                pending = nxt
                # Spread the NEXT group's transposes across this group's
                # pair loop (data is prefetched) so no transpose block sits
                # between the last PV of g and the first QK of g+1.
                spread = list(transpose_units(g + 1)) if (nxt and g >= 1) else []
                transposed_early = bool(spread)
                pending_stages = nxt

                # ---- score + softmax + PV over k-tile pairs ----
                # One PSUM tile per (q-tile, d-half) so each bank is released
                # as soon as its own normalize-read completes.
                opv = [
                    [
                        psp.tile(
                            [P, DH], f32, tag=f"pv{j}_{dh}", bufs=1,
                            name=f"opv{j}_{dh}",
                        )
                        for dh in range(NDH)
                    ]
                    for j in range(QGT)
                ]
                # Running fp32 sum of P^T across this group's pairs (DVE).
                psum_p = psumsb.tile([P, 2, QG], f32, tag="psum_p")
                for p in range(g + 1):
                    diag = p == g
                    stps = psp.tile([P, 2, QG], f32, tag="st", bufs=3)
                    for kk in range(2):
                        ki = 2 * p + kk
                        # Diagonal pair, second k-tile: q < 128 (rel) is fully
                        # masked, so only compute the upper q half (N=128).
                        qlo = P if (diag and kk == 1) else 0
                        for ds in range(ND):
                            nc.tensor.matmul(
                                stps[:, kk, qlo:],
                                lhsT=KT[:, ds, ki * P : (ki + 1) * P],
                                rhs=QT[:, ds, g * QG + qlo : (g + 1) * QG],
                                start=(ds == 0),
                                stop=(ds == ND - 1),
                            )
                    if diag:
                        # The uncomputed quarter never got written: give it a
                        # finite value; the multiplicative mask below zeroes
                        # it (and all other masked entries) after exp.
                        nc.vector.memset(stps[:, 1, :P], 0.0)
                    ptt = ptp.tile([P, 2, QG], bf16, tag="pt")
                    nc.scalar.activation(
                        ptt[:], stps[:], mybir.ActivationFunctionType.Exp,
                        scale=scale,
                    )
                    if diag:
                        nc.vector.tensor_mul(ptt[:], ptt[:], mask01[:])
                    if p == 0:
                        nc.vector.tensor_copy(psum_p[:], ptt[:])
                    else:
                        nc.vector.tensor_add(psum_p[:], psum_p[:], ptt[:])
                    for kk in range(2):
                        ki = 2 * p + kk
                        first = (p == 0) and (kk == 0)
                        for j in range(QGT):
                            if diag and kk == 1 and j == 0:
                                continue  # fully masked block
                            # last matmul touching opv[j]'s accumulation:
                            last_j = diag and (kk == 1 or (kk == 0 and j == 0))
                            lh = ptt[:, kk, j * P : (j + 1) * P]
                            for dh in range(NDH):
                                nc.tensor.matmul(
                                    opv[j][dh][:],
                                    lhsT=lh,
                                    rhs=V[:, ki, dh * DH : (dh + 1) * DH],
                                    start=first,
                                    stop=last_j,
                                )
                    if spread and 1 <= p <= len(spread):
                        nm_, dst_, t_ = spread[p - 1]
                        transpose_unit(nm_, dst_, t_, pending_stages)
                        if p == len(spread):
                            spread = []

                # Any units the pair loop didn't reach (short groups) are
                # emitted here so every tile is transposed exactly once.
                for u, (nm_, dst_, t_) in enumerate(spread):
                    if u >= g:  # pairs 1..g consumed the first g units
                        transpose_unit(nm_, dst_, t_, pending_stages)
                spread = []

                # ---- row sums -> reciprocal -> normalize + store (per
                # d-half; final group splits across DVE+ACT since no later
                # exp can be delayed) ----
                folded = psumsb.tile([P, QG], bf16, tag="folded")
                nc.vector.tensor_add(
                    folded[:], psum_p[:, 0, :], psum_p[:, 1, :]
                )
                rsps = psp.tile([P, QGT], f32, tag="rs", bufs=1)
                for j in range(QGT):
                    nc.tensor.matmul(
                        rsps[:, j : j + 1],
                        lhsT=folded[:, j * P : (j + 1) * P],
                        rhs=ones[:],
                        start=(j == 0),
                        stop=(j == QGT - 1),
                    )
                rec = smallp.tile([P, QGT], f32, tag="rec")
                nc.vector.reciprocal(rec[:], rsps[:])
                final = g == NG - 1
                for j in range(QGT):
                    ost = ostp.tile([P, D], bf16, tag="ost")
                    for dh in range(NDH):
                        osl = ost[:, dh * DH : (dh + 1) * DH]
                        if final and dh % 2 == 1:
                            nc.scalar.mul(osl, opv[j][dh][:], mul=rec[:, j : j + 1])
                        else:
                            nc.vector.tensor_scalar_mul(
                                osl, opv[j][dh][:], scalar1=rec[:, j : j + 1]
                            )
                        seng = nc.scalar if (final and dh % 2 == 1) else nc.sync
                        seng.dma_start(
                            ov[:, g * QGT + j, dh * DH : (dh + 1) * DH], osl
                        )

    nc.compile()
    return nc


_NC_CACHE = {}


def _get_nc(S, D):
    if (S, D) not in _NC_CACHE:
        _NC_CACHE[(S, D)] = build_attention_nc(S, D)
    return _NC_CACHE[(S, D)]


def kernel(query, key, value):
    import ml_dtypes
    from concourse.bass_utils import run_bass_kernel_spmd

    bf = ml_dtypes.bfloat16
    query = np.asarray(query).astype(bf)
    key = np.asarray(key).astype(bf)
    value = np.asarray(value).astype(bf)
    B, S, D = query.shape
    nc = _get_nc(S, D)
    in_maps = [
        {
            "query": np.ascontiguousarray(query[i]),
            "key": np.ascontiguousarray(key[i]),
            "value": np.ascontiguousarray(value[i]),
        }
        for i in range(B)
    ]
    res = run_bass_kernel_spmd(nc, in_maps, core_ids=list(range(B)))
    out = np.stack([r["out"] for r in res.results], axis=0)
    return out.astype(np.float32)

